# revision 10
# baseline (speedup 1.0000x reference)
"""Bayesian dense layer (per-sample reparameterized weights) on 8 TRN2 NeuronCores.

Computes out[b] = x[b] @ (W[b] * softplus(log_std) + mean) + bias for
B=512, IN=OUT=1024, data-parallel over the batch axis (64 rows per core).

Device algorithm per core (batch slice of BPC=64 rows):
  - layout: partition dim = i (contraction index), free dim = o
  - S = softplus(log_std) is precomputed on host (elementwise prep); S and
    mean are kept resident in SBUF; x arrives pre-transposed as xT [IN, BPC]
  - each batch row owns a [1, OUT] PSUM accumulator (base partition 0 —
    matmul outputs must start at partition 0/32/64):
      psum_b = sum_ib xT[:,ib,b].T @ mean_block(ib)   (x @ mean)
             + ones[1,1].T @ bias                      (bias broadcast)
             + sum_ib xT[:,ib,b].T @ (W_tile * S)      (per-sample term)
    where W tiles [128, 2, OUT] stream from HBM (two rows per DMA) and the
    S multiply runs on DVE in place.
The kernel is HBM-bound: it streams 256 MiB of W per core (~750 us at
~358 GB/s per-core HBM bandwidth).
"""

import os
import sys

for _p in ("/root/.axon_site", "/root/.axon_site/_ro/trn_rl_repo",
           "/root/.axon_site/_ro/pypackages"):
    if os.path.isdir(_p) and _p not in sys.path:
        sys.path.append(_p)

import numpy as np

import concourse.bass as bass
import concourse.mybir as mybir
import concourse.tile as tile
from concourse import bacc
from concourse.bass_utils import run_bass_kernel_spmd

B, IN, OUT = 512, 1024, 1024
NCORES = 8
BPC = B // NCORES  # batch rows per core
PAIR = 2           # batch rows fetched per W DMA tile

_BUILT = {}


def build_bass(bpc=BPC, in_dim=IN, out_dim=OUT, pair=PAIR):
    """Build the per-core Bass module (all cores run the same program)."""
    key = (bpc, in_dim, out_dim, pair)
    if key in _BUILT:
        return _BUILT[key]

    f32 = mybir.dt.float32
    nib = in_dim // 128           # i-blocks of 128 partitions
    nch = max(1, out_dim // 512)  # output chunks per matmul (N<=512)
    chunk = out_dim // nch
    npair = bpc // pair

    nc = bacc.Bacc("TRN2", target_bir_lowering=False, debug=False,
                   num_devices=NCORES)

    xT = nc.dram_tensor("xT", [in_dim, bpc], f32, kind="ExternalInput").ap()
    W = nc.dram_tensor("W", [bpc, in_dim, out_dim], f32,
                       kind="ExternalInput").ap()
    S = nc.dram_tensor("S", [in_dim, out_dim], f32, kind="ExternalInput").ap()
    mean = nc.dram_tensor("mean", [in_dim, out_dim], f32,
                          kind="ExternalInput").ap()
    bias = nc.dram_tensor("bias", [1, out_dim], f32, kind="ExternalInput").ap()
    out = nc.dram_tensor("out", [bpc, out_dim], f32,
                         kind="ExternalOutput").ap()

    with tile.TileContext(nc) as tc:
        with (
            tc.tile_pool(name="singles", bufs=1) as singles,
            tc.tile_pool(name="wpool", bufs=4) as wpool,
            tc.tile_pool(name="opool", bufs=4) as opool,
            tc.tile_pool(name="psum", bufs=4, space="PSUM") as psum,
        ):
            xT_sb = singles.tile([128, nib, bpc], f32)
            nc.sync.dma_start(out=xT_sb,
                              in_=xT.rearrange("(ib p) b -> p ib b", p=128))
            S_sb = singles.tile([128, nib, out_dim], f32)
            nc.sync.dma_start(out=S_sb,
                              in_=S.rearrange("(ib p) o -> p ib o", p=128))
            mean_sb = singles.tile([128, nib, out_dim], f32)
            nc.sync.dma_start(out=mean_sb,
                              in_=mean.rearrange("(ib p) o -> p ib o", p=128))
            bias_sb = singles.tile([1, out_dim], f32)
            nc.sync.dma_start(out=bias_sb, in_=bias)
            ones = singles.tile([1, 1], f32)
            nc.vector.memset(ones, 1.0)

            for bp in range(npair):
                accs = []
                for j in range(pair):
                    b = bp * pair + j
                    acc = psum.tile([1, out_dim], f32, tag="acc")
                    accs.append(acc)
                    # x @ mean contribution for this row, initializes PSUM
                    for ib in range(nib):
                        for n in range(nch):
                            nc.tensor.matmul(
                                acc[:, n * chunk:(n + 1) * chunk],
                                xT_sb[:, ib, b:b + 1],
                                mean_sb[:, ib, n * chunk:(n + 1) * chunk],
                                start=(ib == 0), stop=False,
                                skip_group_check=True)
                    # + bias via a K=1 matmul against a ones scalar
                    for n in range(nch):
                        nc.tensor.matmul(
                            acc[:, n * chunk:(n + 1) * chunk],
                            ones,
                            bias_sb[:, n * chunk:(n + 1) * chunk],
                            start=False, stop=False, skip_group_check=True)

                # per-sample term: stream W, scale by S on DVE, contract on PE
                for ib in range(nib):
                    w_t = wpool.tile([128, pair, out_dim], f32)
                    nc.sync.dma_start(
                        out=w_t,
                        in_=W[bp * pair:(bp + 1) * pair,
                              ib * 128:(ib + 1) * 128, :]
                        .rearrange("b p o -> p b o"))
                    for j in range(pair):
                        b = bp * pair + j
                        nc.vector.tensor_mul(w_t[:, j, :], w_t[:, j, :],
                                             S_sb[:, ib, :])
                        for n in range(nch):
                            nc.tensor.matmul(
                                accs[j][:, n * chunk:(n + 1) * chunk],
                                xT_sb[:, ib, b:b + 1],
                                w_t[:, j, n * chunk:(n + 1) * chunk],
                                start=False, stop=(ib == nib - 1),
                                skip_group_check=True)

                for j in range(pair):
                    b = bp * pair + j
                    row = opool.tile([1, out_dim], f32, tag="row")
                    nc.scalar.copy(row, accs[j])
                    nc.sync.dma_start(out=out[b:b + 1, :], in_=row)

    nc.finalize()
    _BUILT[key] = nc
    return nc


def _softplus(x):
    return np.logaddexp(0.0, x.astype(np.float32)).astype(np.float32)


def _run(x, W, mean, log_std, bias, **kwargs):
    x = np.ascontiguousarray(x, dtype=np.float32)
    W = np.ascontiguousarray(W, dtype=np.float32)
    mean = np.ascontiguousarray(mean, dtype=np.float32)
    bias2 = np.ascontiguousarray(bias, dtype=np.float32).reshape(1, OUT)
    S = _softplus(log_std)

    nc = build_bass()
    in_maps = []
    for c in range(NCORES):
        sl = slice(c * BPC, (c + 1) * BPC)
        in_maps.append({
            "xT": np.ascontiguousarray(x[sl].T),
            "W": W[sl],
            "S": S,
            "mean": mean,
            "bias": bias2,
        })
    res = run_bass_kernel_spmd(nc, in_maps, core_ids=list(range(NCORES)),
                               **kwargs)
    out = np.concatenate([res.results[c]["out"] for c in range(NCORES)],
                         axis=0)
    return out, res


def kernel(x, W, mean, log_std, bias):
    return _run(x, W, mean, log_std, bias)[0]


# revision 13
# speedup vs baseline: 2.1376x; 2.1376x over previous
"""Bayesian dense layer (per-sample reparameterized weights) on 8 TRN2 NeuronCores.

Computes out[b] = x[b] @ (W[b] * softplus(log_std) + mean) + bias for
B=512, IN=OUT=1024, data-parallel over the batch axis (64 rows per core).

Device algorithm per core (batch slice of BPC=64 rows):
  - layout: partition dim = i (contraction index), free dim = o
  - S = softplus(log_std) is precomputed on host (elementwise prep) and kept
    resident in SBUF; x arrives pre-transposed as xT [IN, BPC] in both fp32
    (mean term) and bf16 (per-sample term)
  - mean term: psum_mean[64, OUT] = xT.T @ mean + ones.T @ bias, computed
    once at full PE width in fp32 (~15 us), copied to SBUF
  - per-sample term: stream W tiles [128, 2, OUT] from HBM; DVE multiplies
    by S writing bf16; per row b a [1, OUT] PSUM accumulator (matmul outputs
    must start at partition 0/32/64) collects 8 bf16 matmuls
    psum_b += xT_bf16[:, ib, b].T @ (W*S)_bf16  (bf16 streams the PE at
    1 col/cycle vs 1/4 for fp32); rows land back at partition b of an SBUF
    collector via small SBUF->SBUF DMAs
  - merge: one DVE add of the two [64, OUT] terms, one DMA to DRAM
The kernel is HBM-bound: it streams 256 MiB of W per core (~750 us at
~358 GB/s per-core HBM bandwidth).
"""

import os
import sys

for _p in ("/root/.axon_site", "/root/.axon_site/_ro/trn_rl_repo",
           "/root/.axon_site/_ro/pypackages"):
    if os.path.isdir(_p) and _p not in sys.path:
        sys.path.append(_p)

import numpy as np

import concourse.bass as bass
import concourse.mybir as mybir
import concourse.tile as tile
from concourse import bacc
from concourse.bass_utils import run_bass_kernel_spmd

B, IN, OUT = 512, 1024, 1024
NCORES = 8
BPC = B // NCORES  # batch rows per core
PAIR = 2           # batch rows fetched per W DMA tile

_BUILT = {}


def build_bass(bpc=BPC, in_dim=IN, out_dim=OUT, pair=PAIR):
    """Build the per-core Bass module (all cores run the same program)."""
    key = (bpc, in_dim, out_dim, pair)
    if key in _BUILT:
        return _BUILT[key]

    f32 = mybir.dt.float32
    bf16 = mybir.dt.bfloat16
    nib = in_dim // 128           # i-blocks of 128 partitions
    nch = max(1, out_dim // 512)  # output chunks per matmul (N<=512)
    chunk = out_dim // nch
    npair = bpc // pair

    nc = bacc.Bacc("TRN2", target_bir_lowering=False, debug=False,
                   num_devices=NCORES)

    xT = nc.dram_tensor("xT", [in_dim, bpc], f32, kind="ExternalInput").ap()
    xTh = nc.dram_tensor("xTh", [in_dim, bpc], bf16, kind="ExternalInput").ap()
    W = nc.dram_tensor("W", [bpc, in_dim, out_dim], f32,
                       kind="ExternalInput").ap()
    S = nc.dram_tensor("S", [in_dim, out_dim], f32, kind="ExternalInput").ap()
    mean = nc.dram_tensor("mean", [in_dim, out_dim], f32,
                          kind="ExternalInput").ap()
    bias = nc.dram_tensor("bias", [1, out_dim], f32, kind="ExternalInput").ap()
    out = nc.dram_tensor("out", [bpc, out_dim], f32,
                         kind="ExternalOutput").ap()

    with tile.TileContext(nc) as tc:
        with (
            tc.tile_pool(name="singles", bufs=1) as singles,
            tc.tile_pool(name="mpool", bufs=2) as mpool,
            tc.tile_pool(name="wpool", bufs=4) as wpool,
            tc.tile_pool(name="hpool", bufs=4) as hpool,
            tc.tile_pool(name="opool", bufs=4) as opool,
            tc.tile_pool(name="psum", bufs=1, space="PSUM") as psum,
            tc.tile_pool(name="psrow", bufs=3, space="PSUM") as psrow,
        ):
            xT_sb = singles.tile([128, nib, bpc], f32)
            nc.sync.dma_start(out=xT_sb,
                              in_=xT.rearrange("(ib p) b -> p ib b", p=128))
            xTh_sb = singles.tile([128, nib, bpc], bf16)
            nc.sync.dma_start(out=xTh_sb,
                              in_=xTh.rearrange("(ib p) b -> p ib b", p=128))
            S_sb = singles.tile([128, nib, out_dim], f32)
            nc.sync.dma_start(out=S_sb,
                              in_=S.rearrange("(ib p) o -> p ib o", p=128))
            bias_sb = singles.tile([1, out_dim], f32)
            nc.sync.dma_start(out=bias_sb, in_=bias)
            ones = singles.tile([1, bpc], f32)
            nc.vector.memset(ones, 1.0)

            # ── mean term at full PE width: mb_sb = xT.T @ mean + bias ──
            acc_m = psum.tile([bpc, out_dim], f32)
            for ib in range(nib):
                m_t = mpool.tile([128, out_dim], f32)
                nc.sync.dma_start(out=m_t,
                                  in_=mean[ib * 128:(ib + 1) * 128, :])
                for n in range(nch):
                    nc.tensor.matmul(
                        acc_m[:, n * chunk:(n + 1) * chunk],
                        xT_sb[:, ib, :],
                        m_t[:, n * chunk:(n + 1) * chunk],
                        start=(ib == 0), stop=False, skip_group_check=True)
            for n in range(nch):
                nc.tensor.matmul(
                    acc_m[:, n * chunk:(n + 1) * chunk],
                    ones,
                    bias_sb[:, n * chunk:(n + 1) * chunk],
                    start=False, stop=True, skip_group_check=True)
            mb_sb = singles.tile([bpc, out_dim], f32)
            nc.scalar.copy(mb_sb, acc_m)

            # ── per-sample term, collected per row into wt_sb ──
            wt_sb = singles.tile([bpc, out_dim], f32)
            for bp in range(npair):
                accs = [psrow.tile([1, out_dim], f32, tag="acc",
                                   name=f"acc{bp}_{j}")
                        for j in range(pair)]
                for ib in range(nib):
                    w_t = wpool.tile([128, pair, out_dim], f32)
                    nc.sync.dma_start(
                        out=w_t,
                        in_=W[bp * pair:(bp + 1) * pair,
                              ib * 128:(ib + 1) * 128, :]
                        .rearrange("b p o -> p b o"))
                    w_h = hpool.tile([128, pair, out_dim], bf16)
                    for j in range(pair):
                        b = bp * pair + j
                        nc.vector.tensor_mul(w_h[:, j, :], w_t[:, j, :],
                                             S_sb[:, ib, :])
                        for n in range(nch):
                            nc.tensor.matmul(
                                accs[j][:, n * chunk:(n + 1) * chunk],
                                xTh_sb[:, ib, b:b + 1],
                                w_h[:, j, n * chunk:(n + 1) * chunk],
                                start=(ib == 0), stop=(ib == nib - 1),
                                skip_group_check=True)
                for j in range(pair):
                    b = bp * pair + j
                    row = opool.tile([1, out_dim], f32, tag="row")
                    nc.scalar.copy(row, accs[j])
                    nc.sync.dma_start(out=wt_sb[b:b + 1, :], in_=row)

            # ── merge and write out ──
            nc.vector.tensor_add(wt_sb, wt_sb, mb_sb)
            nc.sync.dma_start(out=out, in_=wt_sb)

    nc.finalize()
    _BUILT[key] = nc
    return nc


def _softplus(x):
    return np.logaddexp(0.0, x.astype(np.float32)).astype(np.float32)


def _run(x, W, mean, log_std, bias, **kwargs):
    import ml_dtypes
    x = np.ascontiguousarray(x, dtype=np.float32)
    W = np.ascontiguousarray(W, dtype=np.float32)
    mean = np.ascontiguousarray(mean, dtype=np.float32)
    bias2 = np.ascontiguousarray(bias, dtype=np.float32).reshape(1, OUT)
    S = _softplus(log_std)

    nc = build_bass()
    in_maps = []
    for c in range(NCORES):
        sl = slice(c * BPC, (c + 1) * BPC)
        xTc = np.ascontiguousarray(x[sl].T)
        in_maps.append({
            "xT": xTc,
            "xTh": xTc.astype(ml_dtypes.bfloat16),
            "W": W[sl],
            "S": S,
            "mean": mean,
            "bias": bias2,
        })
    res = run_bass_kernel_spmd(nc, in_maps, core_ids=list(range(NCORES)),
                               **kwargs)
    out = np.concatenate([res.results[c]["out"] for c in range(NCORES)],
                         axis=0)
    return out, res


def kernel(x, W, mean, log_std, bias):
    return _run(x, W, mean, log_std, bias)[0]


# revision 14
# speedup vs baseline: 2.2426x; 1.0491x over previous
"""Bayesian dense layer (per-sample reparameterized weights) on 8 TRN2 NeuronCores.

Computes out[b] = x[b] @ (W[b] * softplus(log_std) + mean) + bias for
B=512, IN=OUT=1024, data-parallel over the batch axis (64 rows per core).

Device algorithm per core (batch slice of BPC=64 rows):
  - layout: partition dim = i (contraction index), free dim = o
  - S = softplus(log_std) is precomputed on host (elementwise prep) and kept
    resident in SBUF; x arrives pre-transposed as xT [IN, BPC] in both fp32
    (mean term) and bf16 (per-sample term)
  - mean term: psum_mean[64, OUT] = xT.T @ mean + ones.T @ bias, computed
    once at full PE width in fp32 (~15 us), copied to SBUF
  - per-sample term: stream W tiles [128, 2, OUT] from HBM; DVE multiplies
    by S writing bf16; per row b a [1, OUT] PSUM accumulator (matmul outputs
    must start at partition 0/32/64) collects 8 bf16 matmuls
    psum_b += xT_bf16[:, ib, b].T @ (W*S)_bf16  (bf16 streams the PE at
    1 col/cycle vs 1/4 for fp32); rows land back at partition b of an SBUF
    collector via small SBUF->SBUF DMAs
  - merge: one DVE add of the two [64, OUT] terms, one DMA to DRAM
The kernel is HBM-bound: it streams 256 MiB of W per core (~750 us at
~358 GB/s per-core HBM bandwidth).
"""

import os
import sys

for _p in ("/root/.axon_site", "/root/.axon_site/_ro/trn_rl_repo",
           "/root/.axon_site/_ro/pypackages"):
    if os.path.isdir(_p) and _p not in sys.path:
        sys.path.append(_p)

import numpy as np

import concourse.bass as bass
import concourse.mybir as mybir
import concourse.tile as tile
from concourse import bacc
from concourse.bass_utils import run_bass_kernel_spmd

B, IN, OUT = 512, 1024, 1024
NCORES = 8
BPC = B // NCORES  # batch rows per core
PAIR = 2           # batch rows fetched per W DMA tile

_BUILT = {}


def build_bass(bpc=BPC, in_dim=IN, out_dim=OUT, pair=PAIR):
    """Build the per-core Bass module (all cores run the same program)."""
    key = (bpc, in_dim, out_dim, pair)
    if key in _BUILT:
        return _BUILT[key]

    f32 = mybir.dt.float32
    bf16 = mybir.dt.bfloat16
    nib = in_dim // 128           # i-blocks of 128 partitions
    nch = max(1, out_dim // 512)  # output chunks per matmul (N<=512)
    chunk = out_dim // nch
    npair = bpc // pair

    nc = bacc.Bacc("TRN2", target_bir_lowering=False, debug=False,
                   num_devices=NCORES)

    xT = nc.dram_tensor("xT", [in_dim, bpc], f32, kind="ExternalInput").ap()
    xTh = nc.dram_tensor("xTh", [in_dim, bpc], bf16, kind="ExternalInput").ap()
    W = nc.dram_tensor("W", [bpc, in_dim, out_dim], f32,
                       kind="ExternalInput").ap()
    S = nc.dram_tensor("S", [in_dim, out_dim], f32, kind="ExternalInput").ap()
    mean = nc.dram_tensor("mean", [in_dim, out_dim], f32,
                          kind="ExternalInput").ap()
    bias = nc.dram_tensor("bias", [1, out_dim], f32, kind="ExternalInput").ap()
    out = nc.dram_tensor("out", [bpc, out_dim], f32,
                         kind="ExternalOutput").ap()

    with tile.TileContext(nc) as tc:
        with (
            tc.tile_pool(name="singles", bufs=1) as singles,
            tc.tile_pool(name="mpool", bufs=2) as mpool,
            tc.tile_pool(name="wpool", bufs=8) as wpool,
            tc.tile_pool(name="hpool", bufs=6) as hpool,
            tc.tile_pool(name="opool", bufs=4) as opool,
            tc.tile_pool(name="psum", bufs=1, space="PSUM") as psum,
            tc.tile_pool(name="psrow", bufs=3, space="PSUM") as psrow,
        ):
            xT_sb = singles.tile([128, nib, bpc], f32)
            nc.sync.dma_start(out=xT_sb,
                              in_=xT.rearrange("(ib p) b -> p ib b", p=128))
            xTh_sb = singles.tile([128, nib, bpc], bf16)
            nc.sync.dma_start(out=xTh_sb,
                              in_=xTh.rearrange("(ib p) b -> p ib b", p=128))
            S_sb = singles.tile([128, nib, out_dim], f32)
            nc.sync.dma_start(out=S_sb,
                              in_=S.rearrange("(ib p) o -> p ib o", p=128))
            bias_sb = singles.tile([1, out_dim], f32)
            nc.sync.dma_start(out=bias_sb, in_=bias)
            ones = singles.tile([1, bpc], f32)
            nc.vector.memset(ones, 1.0)

            # ── mean term at full PE width: mb_sb = xT.T @ mean + bias ──
            acc_m = psum.tile([bpc, out_dim], f32)
            for ib in range(nib):
                m_t = mpool.tile([128, out_dim], f32)
                nc.sync.dma_start(out=m_t,
                                  in_=mean[ib * 128:(ib + 1) * 128, :])
                for n in range(nch):
                    nc.tensor.matmul(
                        acc_m[:, n * chunk:(n + 1) * chunk],
                        xT_sb[:, ib, :],
                        m_t[:, n * chunk:(n + 1) * chunk],
                        start=(ib == 0), stop=False, skip_group_check=True)
            for n in range(nch):
                nc.tensor.matmul(
                    acc_m[:, n * chunk:(n + 1) * chunk],
                    ones,
                    bias_sb[:, n * chunk:(n + 1) * chunk],
                    start=False, stop=True, skip_group_check=True)
            mb_sb = singles.tile([bpc, out_dim], f32)
            nc.scalar.copy(mb_sb, acc_m)

            # ── per-sample term, collected per row into wt_sb ──
            wt_sb = singles.tile([bpc, out_dim], f32)
            for bp in range(npair):
                accs = [psrow.tile([1, out_dim], f32, tag="acc",
                                   name=f"acc{bp}_{j}")
                        for j in range(pair)]
                for ib in range(nib):
                    w_t = wpool.tile([128, pair, out_dim], f32)
                    nc.sync.dma_start(
                        out=w_t,
                        in_=W[bp * pair:(bp + 1) * pair,
                              ib * 128:(ib + 1) * 128, :]
                        .rearrange("b p o -> p b o"))
                    w_h = hpool.tile([128, pair, out_dim], bf16)
                    for j in range(pair):
                        b = bp * pair + j
                        nc.vector.tensor_mul(w_h[:, j, :], w_t[:, j, :],
                                             S_sb[:, ib, :])
                        for n in range(nch):
                            nc.tensor.matmul(
                                accs[j][:, n * chunk:(n + 1) * chunk],
                                xTh_sb[:, ib, b:b + 1],
                                w_h[:, j, n * chunk:(n + 1) * chunk],
                                start=(ib == 0), stop=(ib == nib - 1),
                                skip_group_check=True)
                for j in range(pair):
                    b = bp * pair + j
                    row = opool.tile([1, out_dim], f32, tag="row")
                    nc.scalar.copy(row, accs[j])
                    nc.sync.dma_start(out=wt_sb[b:b + 1, :], in_=row)

            # ── merge and write out ──
            nc.vector.tensor_add(wt_sb, wt_sb, mb_sb)
            nc.sync.dma_start(out=out, in_=wt_sb)

    nc.finalize()
    _BUILT[key] = nc
    return nc


def _softplus(x):
    return np.logaddexp(0.0, x.astype(np.float32)).astype(np.float32)


def _run(x, W, mean, log_std, bias, **kwargs):
    import ml_dtypes
    x = np.ascontiguousarray(x, dtype=np.float32)
    W = np.ascontiguousarray(W, dtype=np.float32)
    mean = np.ascontiguousarray(mean, dtype=np.float32)
    bias2 = np.ascontiguousarray(bias, dtype=np.float32).reshape(1, OUT)
    S = _softplus(log_std)

    nc = build_bass()
    in_maps = []
    for c in range(NCORES):
        sl = slice(c * BPC, (c + 1) * BPC)
        xTc = np.ascontiguousarray(x[sl].T)
        in_maps.append({
            "xT": xTc,
            "xTh": xTc.astype(ml_dtypes.bfloat16),
            "W": W[sl],
            "S": S,
            "mean": mean,
            "bias": bias2,
        })
    res = run_bass_kernel_spmd(nc, in_maps, core_ids=list(range(NCORES)),
                               **kwargs)
    out = np.concatenate([res.results[c]["out"] for c in range(NCORES)],
                         axis=0)
    return out, res


def kernel(x, W, mean, log_std, bias):
    return _run(x, W, mean, log_std, bias)[0]
